# revision 28
# baseline (speedup 1.0000x reference)
"""Distributed multi-head attention (QKV proj + RoPE + softmax attention + out proj)
on 8 TRN2 NeuronCores.

Sharding: tensor-parallel over heads. Core c owns heads (2c, 2c+1):
  - qkv^T = W_c @ x^T for its 384 channels over all 4096 tokens (bf16 matmul)
  - RoPE on q,k (fp32, partition-swap via SBUF-SBUF DMA)
  - scores^T = k @ q^T per (batch, head): both heads' scores go into one
    2-bank PSUM tile (row-tiled concurrent matmuls), one exp [128,1024] on ScalarE
  - ctx^T = [v | 1] @ expS^T : M=65 matmul computes context + softmax denominator
    (ones column baked into the transposed-v layout)
  - denominators batched into one reciprocal per batch; broadcast via a K=1 matmul
  - AllToAll redistributes ctx: head-sharded -> token-sharded (512 tok/core)
  - out^T = W_out^T.T @ ctx_full^T + b_out for the core's 512 tokens

Host side: transposes/shards weights, runs SPMD, gathers [1024, 512] fp32 per core,
transposes to [2, 2048, 1024].
"""

import numpy as np
import ml_dtypes

import concourse.bass as bass
import concourse.tile as tile
from concourse import bacc, mybir
from concourse.bass_utils import run_bass_kernel_spmd
from concourse.masks import make_identity

BF16 = ml_dtypes.bfloat16

B, L, D, H, Hd = 2, 2048, 1024, 16, 64
T = B * L              # 4096 tokens
NC = 8                 # cores
HPC = H // NC          # 2 heads per core
TOK = T // NC          # 512 token shard per core
NT = T // 512          # 8 token n-tiles of 512
KT = L // 128          # 16 k-tiles per batch
QT = L // 512          # 4 q-tiles per batch

F32 = mybir.dt.float32
BF = mybir.dt.bfloat16


def build(debug=False):
    nc = bacc.Bacc(None, target_bir_lowering=False, num_devices=NC)

    xT = nc.dram_tensor("xT", [D, T], BF, kind="ExternalInput")          # x^T, replicated
    wq = nc.dram_tensor("wqkvT", [D, 3 * 128], BF, kind="ExternalInput")  # W_c^T per core
    bq = nc.dram_tensor("bqkv", [128, 3], F32, kind="ExternalInput")      # bias cols q,k,v
    cosT = nc.dram_tensor("cosT", [128, L], F32, kind="ExternalInput")
    sinT = nc.dram_tensor("sinT", [128, L], F32, kind="ExternalInput")    # sign-folded sin
    wo = nc.dram_tensor("woutT", [D, D], BF, kind="ExternalInput")        # W_out^T, replicated
    bo = nc.dram_tensor("bout", [128, NC], F32, kind="ExternalInput")
    out = nc.dram_tensor("out", [D, TOK], F32, kind="ExternalOutput")

    with tile.TileContext(nc) as tc:
        with tc.tile_pool(name="const", bufs=1) as const, \
             tc.tile_pool(name="big", bufs=1) as big, \
             tc.tile_pool(name="rope", bufs=3) as rope, \
             tc.tile_pool(name="es", bufs=8) as esp, \
             tc.tile_pool(name="cu", bufs=10) as cup, \
             tc.tile_pool(name="small", bufs=3) as small, \
             tc.tile_pool(name="psum", bufs=1, space="PSUM") as psum, \
             tc.tile_pool(name="dram", bufs=1, space="DRAM") as dram:

            # ---------------- constants / weights (loaded before x!) ----------
            ident = const.tile([128, 128], BF, tag="ident")
            make_identity(nc, ident[:])
            ones_bc = const.tile([1, 64], F32, tag="ones_bc")
            nc.vector.memset(ones_bc[:], 1.0)

            bq_sb = const.tile([128, 3], F32, tag="bq")
            nc.sync.dma_start(bq_sb[:], bq[:])
            bo_sb = const.tile([128, NC], F32, tag="bo")
            cos_sb = const.tile([128, L], F32, tag="cos")
            nc.sync.dma_start(cos_sb[:], cosT[:])
            sin_sb = const.tile([128, L], F32, tag="sin")
            nc.sync.dma_start(sin_sb[:], sinT[:])
            w_sb = []
            for k in range(8):
                t = big.tile([128, 3 * 128], BF, tag=f"w{k}", name=f"w{k}")
                nc.sync.dma_start(t[:], wq[128 * k:128 * (k + 1), :])
                w_sb.append(t)
            wo_sb = [big.tile([128, D], BF, tag=f"wo{k}", name=f"wo_{k}")
                     for k in range(8)]


            qT_sb = big.tile([128, T], BF, tag="qT")
            kT_sb = big.tile([128, T], BF, tag="kT")
            v_sb = big.tile([128, T], BF, tag="v")
            # transposed v with a built-in ones column: [tok%128, blk, head, 65]
            vn_sb = big.tile([128, T // 128, HPC, 65], BF, tag="vn")
            nc.vector.memset(vn_sb[:, :, :, 64:65], 1.0)

            a2a_in = dram.tile([NC, 128, TOK], BF, tag="a2a_in")
            a2a_out = dram.tile([NC, 128, TOK], BF, tag="a2a_out")

            # ---------------- per-stage emitters ------------------------------
            _xc_cache = {}

            def stage1_load(n):
                ts = slice(512 * n, 512 * (n + 1))
                xc = []
                for k in range(8):
                    t = rope.tile([128, 512], BF, tag="xc", bufs=24,
                                  name=f"xc_{n}_{k}")
                    nc.sync.dma_start(t[:], xT[128 * k:128 * (k + 1), ts])
                    xc.append(t)
                _xc_cache[n] = xc

            def stage1_qkv_m(n, m):
                """QKV matmul + bias (+rope for q/k) for one (n-tile, m)."""
                ts = slice(512 * n, 512 * (n + 1))
                cs = slice(512 * (n % QT), 512 * (n % QT) + 512)
                xc = _xc_cache[n]
                if True:
                    ps = psum.tile([128, 512], F32, tag="st", bufs=3,
                                   name=f"s1_{n}_{m}")
                    for k in range(8):
                        nc.tensor.matmul(
                            ps[:],
                            w_sb[k][:, 128 * m:128 * (m + 1)],
                            xc[k][:],
                            start=(k == 0), stop=(k == 7),
                        )
                    if m < 2:  # q or k: ACT evicts (+bias) fast to free the
                        # PSUM slot; rope split across DVE and GpSimd
                        dst = qT_sb if m == 0 else kT_sb
                        qb = rope.tile([128, 512], F32, tag="qb", bufs=6,
                                       name=f"qb_{n}_{m}")
                        nc.scalar.activation(
                            qb[:], ps[:],
                            mybir.ActivationFunctionType.Identity,
                            bias=bq_sb[:, m:m + 1])
                        qc = rope.tile([128, 512], F32, tag="qc", name=f"qc_{n}_{m}")
                        nc.vector.tensor_tensor(
                            qc[:], qb[:], cos_sb[:, cs], mybir.AluOpType.mult)
                        qs = rope.tile([128, 512], F32, tag="qs", name=f"qs_{n}_{m}")
                        nc.gpsimd.tensor_tensor(
                            qs[:], qb[:], sin_sb[:, cs], mybir.AluOpType.mult)
                        qw = rope.tile([128, 512], F32, tag="qw", name=f"qw_{n}_{m}")
                        for blk in range(4):
                            src = 32 * (blk ^ 1)
                            nc.gpsimd.dma_start(
                                qw[32 * blk:32 * blk + 32, :],
                                qs[src:src + 32, :])
                        nc.vector.tensor_tensor(
                            dst[:, ts], qc[:], qw[:], mybir.AluOpType.add)
                    else:  # v: bias only, straight to bf16
                        nc.scalar.activation(
                            v_sb[:, ts], ps[:],
                            mybir.ActivationFunctionType.Identity,
                            bias=bq_sb[:, 2:3])

            def stage1_qkv(n):
                stage1_load(n)
                for m in range(3):
                    stage1_qkv_m(n, m)

            def stage1_vtr(j):
                """Transpose one 128-token block of v into vn (both heads)."""
                tp = psum.tile([128, 128], BF, tag="st", bufs=3, name=f"tr_{j}")
                nc.tensor.transpose(tp[:], v_sb[:, 128 * j:128 * (j + 1)], ident[:])
                for h in range(HPC):
                    nc.vector.tensor_copy(
                        vn_sb[:, j, h, 0:64], tp[:, 64 * h:64 * (h + 1)])

            def stage2_open(b, qt):
                return [psum.tile([65, 512], F32, tag=f"ctx{h}", bufs=1,
                                  name=f"ctx_{b}_{qt}_{h}")
                        for h in range(HPC)]

            def stage2_kts(b, qt, ctxs, kts, fill_iter):
                qsl = slice(2048 * b + 512 * qt, 2048 * b + 512 * qt + 512)
                for kt in kts:
                    ksl = slice(2048 * b + 128 * kt, 2048 * b + 128 * kt + 128)
                    blk = 16 * b + kt
                    st2 = psum.tile([128, 1024], F32, tag="st", bufs=3,
                                    name=f"st_{b}_{qt}_{kt}")
                    for h in range(HPC):
                        nc.tensor.matmul(
                            st2[:, 512 * h:512 * (h + 1)],
                            kT_sb[64 * h:64 * (h + 1), ksl],
                            qT_sb[64 * h:64 * (h + 1), qsl],
                            start=True, stop=True)
                    es = esp.tile([128, 1024], BF, tag="es",
                                  name=f"es_{b}_{qt}_{kt}")
                    nc.scalar.activation(
                        es[:], st2[:], mybir.ActivationFunctionType.Exp)
                    for h in range(HPC):
                        nc.tensor.matmul(
                            ctxs[h][:],
                            vn_sb[:, blk, h, :],
                            es[:, 512 * h:512 * (h + 1)],
                            start=(kt == 0), stop=(kt == KT - 1))
                    fill_iter(b, qt, kt)

            def stage2_qtile(b, qt, ctx_evict, fill_iter):
                ctxs = stage2_open(b, qt)
                stage2_kts(b, qt, ctxs, range(KT), fill_iter)
                ctx_evict(qt, ctxs)

            def run_batch(b, fill_iter, qts=range(QT), pre_ctxs=None):
                """Stage-2 for one batch; per-(qt,h) pipelined normalization."""

                def ctx_evict(qt, ctxs):
                    for h in range(HPC):
                        cu = cup.tile([65, 512], F32, tag="cu",
                                      name=f"cu_{b}_{qt}_{h}")
                        nc.vector.tensor_copy(cu[:], ctxs[h][:])
                        dn = small.tile([1, 512], F32, tag="dn",
                                        name=f"dn_{b}_{qt}_{h}", bufs=3)
                        nc.vector.tensor_copy(dn[:], cu[64:65, :])
                        rc = small.tile([1, 512], F32, tag="rc",
                                        name=f"rc_{b}_{qt}_{h}", bufs=3)
                        nc.vector.reciprocal_approx_fast(rc[:], dn[:])
                        bcp = psum.tile([64, 512], F32, tag="st", bufs=3,
                                        name=f"bcp_{b}_{qt}_{h}")
                        nc.tensor.matmul(
                            bcp[:], ones_bc[:], rc[:], start=True, stop=True)
                        cn = small.tile([64, 512], BF, tag="cn",
                                        name=f"cn_{b}_{qt}_{h}")
                        nc.vector.tensor_tensor(
                            cn[:], cu[0:64, :], bcp[:],
                            mybir.AluOpType.mult)
                        nc.sync.dma_start(
                            a2a_in[QT * b + qt, 64 * h:64 * (h + 1), :], cn[:])

                if pre_ctxs is not None:
                    ctx_evict(0, pre_ctxs)
                for qt in qts:
                    stage2_qtile(b, qt, ctx_evict, fill_iter)
                return ctx_evict

            # ---------------- emission schedule -------------------------------
            # stage 1 for batch 0, with (b0, qt0) attention riding along:
            # kt-chunk 4n..4n+3 only needs k/v n-tiles <= n and q n-tile 0
            nofill = lambda b, qt, kt: None
            ctxs_q0 = None
            for n in range(QT):
                stage1_qkv(n)
                for j in range(4 * n, 4 * n + 4):
                    stage1_vtr(j)
                if ctxs_q0 is None:
                    ctxs_q0 = stage2_open(0, 0)
                stage2_kts(0, 0, ctxs_q0, range(4 * n, 4 * n + 4), nofill)

            # stage 2 for batch 0, with stage-1(b=1) units drip-fed to keep PE busy
            b1_units = []
            for n in range(QT, NT):
                b1_units.append(lambda n=n: stage1_load(n))
                for m in range(3):
                    b1_units.append(lambda n=n, m=m: stage1_qkv_m(n, m))
                for j in range(4 * n, 4 * n + 4):
                    b1_units.append(lambda j=j: stage1_vtr(j))
            unit_idx = [0]
            count = [0]
            # 64 kt-iterations in batch 0; 32 fill units -> every 2nd iteration
            def fill_iter(b, qt, kt):
                count[0] += 1
                if b == 0 and count[0] % 2 == 0 and unit_idx[0] < len(b1_units):
                    b1_units[unit_idx[0]]()
                    unit_idx[0] += 1

            run_batch(0, fill_iter, qts=range(1, QT), pre_ctxs=ctxs_q0)
            for k in range(8):
                nc.sync.dma_start(wo_sb[k][:], wo[128 * k:128 * (k + 1), :])
            nc.sync.dma_start(bo_sb[:], bo[:])
            while unit_idx[0] < len(b1_units):
                b1_units[unit_idx[0]]()
                unit_idx[0] += 1
            # dummy matmuls keep the PE HAM-warm through ACT-bound b=1
            dummy_scr = small.tile([1, 512], F32, tag="dscr", name="dscr", bufs=1)
            dummy_cnt = [0]

            def emit_dummies(n):
                for _ in range(n):
                    i = dummy_cnt[0]
                    dummy_cnt[0] += 1
                    if i % 8 == 0:
                        dummy_cnt.append(psum.tile(
                            [128, 512], F32, tag="st", bufs=3,
                            name=f"dmy{i}"))
                    dp = dummy_cnt[-1]
                    nc.tensor.matmul(
                        dp[:], ident[:], kT_sb[:, 0:512],
                        start=(i % 8 == 0), stop=(i % 8 == 7))
                    if i % 8 == 7:
                        nc.vector.tensor_copy(
                            dummy_scr[:, 2 * (i // 8):2 * (i // 8) + 2],
                            dp[0:1, 0:2])

            run_batch(1, lambda b, qt, kt: None)

            # ---------------- stage 3: AllToAll ------------------------------
            nc.gpsimd.collective_compute(
                "AllToAll",
                mybir.AluOpType.bypass,
                replica_groups=[list(range(NC))],
                ins=[a2a_in.opt()],
                outs=[a2a_out.opt()],
            )

            emit_dummies(140)
            dscr_dr = dram.tile([1, 512], F32, tag="dscr_dr", name="dscr_dr")
            nc.sync.dma_start(dscr_dr[:], dummy_scr[:])

            # ---------------- stage 4: out projection ------------------------
            ctxf_sb = []
            for k in range(8):
                t = big.tile([128, TOK], BF, tag=f"cf{k}", name=f"cf{k}")
                nc.sync.dma_start(t[:], a2a_out[k, :, :])
                ctxf_sb.append(t)
            for m in range(8):
                pso = psum.tile([128, TOK], F32, tag="st", bufs=3, name=f"o_{m}")
                for k in range(8):
                    nc.tensor.matmul(
                        pso[:],
                        wo_sb[k][:, 128 * m:128 * (m + 1)],
                        ctxf_sb[k][:],
                        start=(k == 0), stop=(k == 7))
                os_t = small.tile([128, TOK], F32, tag="os", name=f"os_{m}")
                nc.scalar.activation(
                    os_t[:], pso[:],
                    mybir.ActivationFunctionType.Identity,
                    bias=bo_sb[:, m:m + 1])
                (nc.sync if m % 2 == 0 else nc.gpsimd).dma_start(
                    out[128 * m:128 * (m + 1), :], os_t[:])

    nc.compile()
    return nc


_NC_CACHE = None


def _get_nc():
    global _NC_CACHE
    if _NC_CACHE is None:
        _NC_CACHE = build()
    return _NC_CACHE


def _host_prep(x, W_qkv, b_qkv, W_out, b_out):
    x = np.asarray(x, dtype=np.float32)
    W_qkv = np.asarray(W_qkv, dtype=np.float32)
    b_qkv = np.asarray(b_qkv, dtype=np.float32)
    W_out = np.asarray(W_out, dtype=np.float32)
    b_out = np.asarray(b_out, dtype=np.float32)

    scale = 1.0 / np.sqrt(Hd)
    xT = np.ascontiguousarray(x.reshape(T, D).T).astype(BF16)

    # rope tables (token position within batch), channel-transposed + sign-folded
    inv_freq = 1.0 / (10000.0 ** (np.arange(0, Hd, 2, dtype=np.float32) / Hd))  # [32]
    t_pos = np.arange(L, dtype=np.float32)
    freqs = np.outer(t_pos, inv_freq)                       # [L, 32]
    emb = np.concatenate([freqs, freqs], axis=1)            # [L, 64]
    cos_t = np.cos(emb).T.astype(np.float32)                # [64, L]
    sin_t = np.sin(emb).T.astype(np.float32)                # [64, L]
    sin2 = sin_t.copy()
    sin2[32:, :] *= -1.0                                    # s''[d] = +sin d<32, -sin d>=32
    cosT = np.ascontiguousarray(np.tile(cos_t, (2, 1)))     # [128, L]
    sinT = np.ascontiguousarray(np.tile(sin2, (2, 1)))

    woutT = np.ascontiguousarray(W_out.T).astype(BF16)      # [D, D]
    bo_sb = np.ascontiguousarray(b_out.reshape(NC, 128).T)  # [128, 8]

    in_maps = []
    for c in range(NC):
        r = slice(128 * c, 128 * (c + 1))
        Wq = W_qkv[0 * D:1 * D][r] * scale
        Wk = W_qkv[1 * D:2 * D][r]
        Wv = W_qkv[2 * D:3 * D][r]
        Wc = np.concatenate([Wq, Wk, Wv], axis=0)           # [384, 1024]
        WcT = np.ascontiguousarray(Wc.T).astype(BF16)       # [1024, 384]
        bq_c = np.stack([
            b_qkv[0 * D:1 * D][r] * scale,
            b_qkv[1 * D:2 * D][r],
            b_qkv[2 * D:3 * D][r],
        ], axis=1).astype(np.float32)                       # [128, 3]
        in_maps.append({
            "xT": xT,
            "wqkvT": WcT,
            "bqkv": np.ascontiguousarray(bq_c),
            "cosT": cosT,
            "sinT": sinT,
            "woutT": woutT,
            "bout": bo_sb,
        })
    return in_maps


def kernel_run(inputs, trace=False, tmpdir=None):
    nc = _get_nc()
    in_maps = _host_prep(**inputs)
    res = run_bass_kernel_spmd(
        nc, in_maps, list(range(NC)), trace=trace, tmpdir=tmpdir)
    outT = np.concatenate(
        [np.asarray(res.results[c]["out"], dtype=np.float32) for c in range(NC)],
        axis=1)                                             # [1024, 4096]
    out = np.ascontiguousarray(outT.T).reshape(B, L, D)
    return out, res


def kernel(**inputs):
    out, _ = kernel_run(inputs, trace=False)
    return out


# revision 29
# speedup vs baseline: 1.0658x; 1.0658x over previous
"""Distributed multi-head attention (QKV proj + RoPE + softmax attention + out proj)
on 8 TRN2 NeuronCores.

Sharding: tensor-parallel over heads. Core c owns heads (2c, 2c+1):
  - qkv^T = W_c @ x^T for its 384 channels over all 4096 tokens (bf16 matmul)
  - RoPE on q,k (fp32, partition-swap via SBUF-SBUF DMA)
  - scores^T = k @ q^T per (batch, head): both heads' scores go into one
    2-bank PSUM tile (row-tiled concurrent matmuls), one exp [128,1024] on ScalarE
  - ctx^T = [v | 1] @ expS^T : M=65 matmul computes context + softmax denominator
    (ones column baked into the transposed-v layout)
  - denominators batched into one reciprocal per batch; broadcast via a K=1 matmul
  - AllToAll redistributes ctx: head-sharded -> token-sharded (512 tok/core)
  - out^T = W_out^T.T @ ctx_full^T + b_out for the core's 512 tokens

Host side: transposes/shards weights, runs SPMD, gathers [1024, 512] fp32 per core,
transposes to [2, 2048, 1024].
"""

import numpy as np
import ml_dtypes

import concourse.bass as bass
import concourse.tile as tile
from concourse import bacc, mybir
from concourse.bass_utils import run_bass_kernel_spmd
from concourse.masks import make_identity

BF16 = ml_dtypes.bfloat16

B, L, D, H, Hd = 2, 2048, 1024, 16, 64
T = B * L              # 4096 tokens
NC = 8                 # cores
HPC = H // NC          # 2 heads per core
TOK = T // NC          # 512 token shard per core
NT = T // 512          # 8 token n-tiles of 512
KT = L // 128          # 16 k-tiles per batch
QT = L // 512          # 4 q-tiles per batch

F32 = mybir.dt.float32
BF = mybir.dt.bfloat16


def build(debug=False):
    nc = bacc.Bacc(None, target_bir_lowering=False, num_devices=NC)

    xT = nc.dram_tensor("xT", [D, T], BF, kind="ExternalInput")          # x^T, replicated
    wq = nc.dram_tensor("wqkvT", [D, 3 * 128], BF, kind="ExternalInput")  # W_c^T per core
    bq = nc.dram_tensor("bqkv", [128, 3], F32, kind="ExternalInput")      # bias cols q,k,v
    cosT = nc.dram_tensor("cosT", [128, L], F32, kind="ExternalInput")
    sinT = nc.dram_tensor("sinT", [128, L], F32, kind="ExternalInput")    # sign-folded sin
    wo = nc.dram_tensor("woutT", [D, D], BF, kind="ExternalInput")        # W_out^T, replicated
    bo = nc.dram_tensor("bout", [128, NC], F32, kind="ExternalInput")
    out = nc.dram_tensor("out", [D, TOK], F32, kind="ExternalOutput")

    with tile.TileContext(nc) as tc:
        with tc.tile_pool(name="const", bufs=1) as const, \
             tc.tile_pool(name="big", bufs=1) as big, \
             tc.tile_pool(name="rope", bufs=3) as rope, \
             tc.tile_pool(name="es", bufs=8) as esp, \
             tc.tile_pool(name="cu", bufs=10) as cup, \
             tc.tile_pool(name="small", bufs=3) as small, \
             tc.tile_pool(name="psum", bufs=1, space="PSUM") as psum, \
             tc.tile_pool(name="dram", bufs=1, space="DRAM") as dram:

            # ---------------- constants / weights (loaded before x!) ----------
            ident = const.tile([128, 128], BF, tag="ident")
            make_identity(nc, ident[:])
            ones_bc = const.tile([1, 64], F32, tag="ones_bc")
            nc.vector.memset(ones_bc[:], 1.0)

            bq_sb = const.tile([128, 3], F32, tag="bq")
            nc.sync.dma_start(bq_sb[:], bq[:])
            bo_sb = const.tile([128, NC], F32, tag="bo")
            cos_sb = const.tile([128, L], F32, tag="cos")
            nc.sync.dma_start(cos_sb[:], cosT[:])
            sin_sb = const.tile([128, L], F32, tag="sin")
            nc.sync.dma_start(sin_sb[:], sinT[:])
            w_sb = []
            for k in range(8):
                t = big.tile([128, 3 * 128], BF, tag=f"w{k}", name=f"w{k}")
                nc.sync.dma_start(t[:], wq[128 * k:128 * (k + 1), :])
                w_sb.append(t)
            wo_sb = [big.tile([128, D], BF, tag=f"wo{k}", name=f"wo_{k}")
                     for k in range(8)]


            qT_sb = big.tile([128, T], BF, tag="qT")
            kT_sb = big.tile([128, T], BF, tag="kT")
            v_sb = big.tile([128, T], BF, tag="v")
            # transposed v with a built-in ones column: [tok%128, blk, head, 65]
            vn_sb = big.tile([128, T // 128, HPC, 65], BF, tag="vn")
            nc.vector.memset(vn_sb[:, :, :, 64:65], 1.0)

            a2a_in = dram.tile([NC, 128, TOK], BF, tag="a2a_in")
            a2a_out = dram.tile([NC, 128, TOK], BF, tag="a2a_out")

            # ---------------- per-stage emitters ------------------------------
            _xc_cache = {}

            def stage1_load(n):
                ts = slice(512 * n, 512 * (n + 1))
                xc = []
                for k in range(8):
                    t = rope.tile([128, 512], BF, tag="xc", bufs=24,
                                  name=f"xc_{n}_{k}")
                    nc.sync.dma_start(t[:], xT[128 * k:128 * (k + 1), ts])
                    xc.append(t)
                _xc_cache[n] = xc

            def stage1_qkv_m(n, m):
                """QKV matmul + bias (+rope for q/k) for one (n-tile, m)."""
                ts = slice(512 * n, 512 * (n + 1))
                cs = slice(512 * (n % QT), 512 * (n % QT) + 512)
                xc = _xc_cache[n]
                if True:
                    ps = psum.tile([128, 512], F32, tag="st", bufs=3,
                                   name=f"s1_{n}_{m}")
                    for k in range(8):
                        nc.tensor.matmul(
                            ps[:],
                            w_sb[k][:, 128 * m:128 * (m + 1)],
                            xc[k][:],
                            start=(k == 0), stop=(k == 7),
                        )
                    if m < 2:  # q or k: ACT evicts (+bias) fast to free the
                        # PSUM slot; rope split across DVE and GpSimd
                        dst = qT_sb if m == 0 else kT_sb
                        qb = rope.tile([128, 512], F32, tag="qb", bufs=6,
                                       name=f"qb_{n}_{m}")
                        nc.scalar.activation(
                            qb[:], ps[:],
                            mybir.ActivationFunctionType.Identity,
                            bias=bq_sb[:, m:m + 1])
                        qc = rope.tile([128, 512], F32, tag="qc", name=f"qc_{n}_{m}")
                        nc.vector.tensor_tensor(
                            qc[:], qb[:], cos_sb[:, cs], mybir.AluOpType.mult)
                        qs = rope.tile([128, 512], F32, tag="qs", name=f"qs_{n}_{m}")
                        nc.gpsimd.tensor_tensor(
                            qs[:], qb[:], sin_sb[:, cs], mybir.AluOpType.mult)
                        qw = rope.tile([128, 512], F32, tag="qw", name=f"qw_{n}_{m}")
                        for blk in range(4):
                            src = 32 * (blk ^ 1)
                            nc.gpsimd.dma_start(
                                qw[32 * blk:32 * blk + 32, :],
                                qs[src:src + 32, :])
                        nc.vector.tensor_tensor(
                            dst[:, ts], qc[:], qw[:], mybir.AluOpType.add)
                    else:  # v: bias only, straight to bf16
                        nc.scalar.activation(
                            v_sb[:, ts], ps[:],
                            mybir.ActivationFunctionType.Identity,
                            bias=bq_sb[:, 2:3])

            def stage1_qkv(n):
                stage1_load(n)
                for m in range(3):
                    stage1_qkv_m(n, m)

            def stage1_vtr(j):
                """Transpose one 128-token block of v into vn (both heads)."""
                tp = psum.tile([128, 128], BF, tag="st", bufs=3, name=f"tr_{j}")
                nc.tensor.transpose(tp[:], v_sb[:, 128 * j:128 * (j + 1)], ident[:])
                for h in range(HPC):
                    nc.vector.tensor_copy(
                        vn_sb[:, j, h, 0:64], tp[:, 64 * h:64 * (h + 1)])

            def stage2_open(b, qt):
                return [psum.tile([65, 512], F32, tag=f"ctx{h}", bufs=1,
                                  name=f"ctx_{b}_{qt}_{h}")
                        for h in range(HPC)]

            def stage2_kts(b, qt, ctxs, kts, fill_iter):
                qsl = slice(2048 * b + 512 * qt, 2048 * b + 512 * qt + 512)
                for kt in kts:
                    ksl = slice(2048 * b + 128 * kt, 2048 * b + 128 * kt + 128)
                    blk = 16 * b + kt
                    st2 = psum.tile([128, 1024], F32, tag="st", bufs=3,
                                    name=f"st_{b}_{qt}_{kt}")
                    for h in range(HPC):
                        nc.tensor.matmul(
                            st2[:, 512 * h:512 * (h + 1)],
                            kT_sb[64 * h:64 * (h + 1), ksl],
                            qT_sb[64 * h:64 * (h + 1), qsl],
                            start=True, stop=True)
                    es = esp.tile([128, 1024], BF, tag="es",
                                  name=f"es_{b}_{qt}_{kt}")
                    nc.scalar.activation(
                        es[:], st2[:], mybir.ActivationFunctionType.Exp)
                    for h in range(HPC):
                        nc.tensor.matmul(
                            ctxs[h][:],
                            vn_sb[:, blk, h, :],
                            es[:, 512 * h:512 * (h + 1)],
                            start=(kt == 0), stop=(kt == KT - 1))
                    fill_iter(b, qt, kt)

            def stage2_qtile(b, qt, ctx_evict, fill_iter):
                ctxs = stage2_open(b, qt)
                stage2_kts(b, qt, ctxs, range(KT), fill_iter)
                ctx_evict(qt, ctxs)

            def run_batch(b, fill_iter, qts=range(QT), pre_ctxs=None):
                """Stage-2 for one batch; per-(qt,h) pipelined normalization."""

                def ctx_evict(qt, ctxs):
                    for h in range(HPC):
                        cu = cup.tile([65, 512], F32, tag="cu",
                                      name=f"cu_{b}_{qt}_{h}")
                        nc.vector.tensor_copy(cu[:], ctxs[h][:])
                        dn = small.tile([1, 512], F32, tag="dn",
                                        name=f"dn_{b}_{qt}_{h}", bufs=3)
                        nc.vector.tensor_copy(dn[:], cu[64:65, :])
                        rc = small.tile([1, 512], F32, tag="rc",
                                        name=f"rc_{b}_{qt}_{h}", bufs=3)
                        nc.vector.reciprocal_approx_fast(rc[:], dn[:])
                        bcp = psum.tile([64, 512], F32, tag="st", bufs=3,
                                        name=f"bcp_{b}_{qt}_{h}")
                        nc.tensor.matmul(
                            bcp[:], ones_bc[:], rc[:], start=True, stop=True)
                        cn = small.tile([64, 512], BF, tag="cn",
                                        name=f"cn_{b}_{qt}_{h}")
                        nc.vector.tensor_tensor(
                            cn[:], cu[0:64, :], bcp[:],
                            mybir.AluOpType.mult)
                        nc.sync.dma_start(
                            a2a_in[QT * b + qt, 64 * h:64 * (h + 1), :], cn[:])

                if pre_ctxs is not None:
                    ctx_evict(0, pre_ctxs)
                for qt in qts:
                    stage2_qtile(b, qt, ctx_evict, fill_iter)
                return ctx_evict

            # ---------------- emission schedule -------------------------------
            # stage 1 for batch 0 (transposes follow each n-tile's v)
            for n in range(QT):
                stage1_qkv(n)
                for j in range(4 * n, 4 * n + 4):
                    stage1_vtr(j)

            # stage 2 for batch 0, with stage-1(b=1) units drip-fed to keep PE busy
            b1_units = []
            for n in range(QT, NT):
                b1_units.append(lambda n=n: stage1_load(n))
                for m in range(3):
                    b1_units.append(lambda n=n, m=m: stage1_qkv_m(n, m))
                for j in range(4 * n, 4 * n + 4):
                    b1_units.append(lambda j=j: stage1_vtr(j))
            unit_idx = [0]
            count = [0]
            # 64 kt-iterations in batch 0; 32 fill units -> every 2nd iteration
            def fill_iter(b, qt, kt):
                count[0] += 1
                if b == 0 and count[0] % 2 == 0 and unit_idx[0] < len(b1_units):
                    b1_units[unit_idx[0]]()
                    unit_idx[0] += 1

            run_batch(0, fill_iter)
            for k in range(8):
                nc.sync.dma_start(wo_sb[k][:], wo[128 * k:128 * (k + 1), :])
            nc.sync.dma_start(bo_sb[:], bo[:])
            while unit_idx[0] < len(b1_units):
                b1_units[unit_idx[0]]()
                unit_idx[0] += 1
            # dummy matmuls keep the PE HAM-warm through ACT-bound b=1
            dummy_scr = small.tile([1, 512], F32, tag="dscr", name="dscr", bufs=1)
            dummy_cnt = [0]

            def emit_dummies(n):
                for _ in range(n):
                    i = dummy_cnt[0]
                    dummy_cnt[0] += 1
                    if i % 8 == 0:
                        dummy_cnt.append(psum.tile(
                            [128, 512], F32, tag="st", bufs=3,
                            name=f"dmy{i}"))
                    dp = dummy_cnt[-1]
                    nc.tensor.matmul(
                        dp[:], ident[:], kT_sb[:, 0:512],
                        start=(i % 8 == 0), stop=(i % 8 == 7))
                    if i % 8 == 7:
                        nc.vector.tensor_copy(
                            dummy_scr[:, 2 * (i // 8):2 * (i // 8) + 2],
                            dp[0:1, 0:2])

            run_batch(1, lambda b, qt, kt: None)

            # ---------------- stage 3: AllToAll ------------------------------
            nc.gpsimd.collective_compute(
                "AllToAll",
                mybir.AluOpType.bypass,
                replica_groups=[list(range(NC))],
                ins=[a2a_in.opt()],
                outs=[a2a_out.opt()],
            )

            emit_dummies(140)
            dscr_dr = dram.tile([1, 512], F32, tag="dscr_dr", name="dscr_dr")
            nc.sync.dma_start(dscr_dr[:], dummy_scr[:])

            # ---------------- stage 4: out projection ------------------------
            ctxf_sb = []
            for k in range(8):
                t = big.tile([128, TOK], BF, tag=f"cf{k}", name=f"cf{k}")
                nc.sync.dma_start(t[:], a2a_out[k, :, :])
                ctxf_sb.append(t)
            for m in range(8):
                pso = psum.tile([128, TOK], F32, tag="st", bufs=3, name=f"o_{m}")
                for k in range(8):
                    nc.tensor.matmul(
                        pso[:],
                        wo_sb[k][:, 128 * m:128 * (m + 1)],
                        ctxf_sb[k][:],
                        start=(k == 0), stop=(k == 7))
                os_t = small.tile([128, TOK], F32, tag="os", name=f"os_{m}")
                nc.scalar.activation(
                    os_t[:], pso[:],
                    mybir.ActivationFunctionType.Identity,
                    bias=bo_sb[:, m:m + 1])
                (nc.sync if m % 2 == 0 else nc.gpsimd).dma_start(
                    out[128 * m:128 * (m + 1), :], os_t[:])

    nc.compile()
    return nc


_NC_CACHE = None


def _get_nc():
    global _NC_CACHE
    if _NC_CACHE is None:
        _NC_CACHE = build()
    return _NC_CACHE


def _host_prep(x, W_qkv, b_qkv, W_out, b_out):
    x = np.asarray(x, dtype=np.float32)
    W_qkv = np.asarray(W_qkv, dtype=np.float32)
    b_qkv = np.asarray(b_qkv, dtype=np.float32)
    W_out = np.asarray(W_out, dtype=np.float32)
    b_out = np.asarray(b_out, dtype=np.float32)

    scale = 1.0 / np.sqrt(Hd)
    xT = np.ascontiguousarray(x.reshape(T, D).T).astype(BF16)

    # rope tables (token position within batch), channel-transposed + sign-folded
    inv_freq = 1.0 / (10000.0 ** (np.arange(0, Hd, 2, dtype=np.float32) / Hd))  # [32]
    t_pos = np.arange(L, dtype=np.float32)
    freqs = np.outer(t_pos, inv_freq)                       # [L, 32]
    emb = np.concatenate([freqs, freqs], axis=1)            # [L, 64]
    cos_t = np.cos(emb).T.astype(np.float32)                # [64, L]
    sin_t = np.sin(emb).T.astype(np.float32)                # [64, L]
    sin2 = sin_t.copy()
    sin2[32:, :] *= -1.0                                    # s''[d] = +sin d<32, -sin d>=32
    cosT = np.ascontiguousarray(np.tile(cos_t, (2, 1)))     # [128, L]
    sinT = np.ascontiguousarray(np.tile(sin2, (2, 1)))

    woutT = np.ascontiguousarray(W_out.T).astype(BF16)      # [D, D]
    bo_sb = np.ascontiguousarray(b_out.reshape(NC, 128).T)  # [128, 8]

    in_maps = []
    for c in range(NC):
        r = slice(128 * c, 128 * (c + 1))
        Wq = W_qkv[0 * D:1 * D][r] * scale
        Wk = W_qkv[1 * D:2 * D][r]
        Wv = W_qkv[2 * D:3 * D][r]
        Wc = np.concatenate([Wq, Wk, Wv], axis=0)           # [384, 1024]
        WcT = np.ascontiguousarray(Wc.T).astype(BF16)       # [1024, 384]
        bq_c = np.stack([
            b_qkv[0 * D:1 * D][r] * scale,
            b_qkv[1 * D:2 * D][r],
            b_qkv[2 * D:3 * D][r],
        ], axis=1).astype(np.float32)                       # [128, 3]
        in_maps.append({
            "xT": xT,
            "wqkvT": WcT,
            "bqkv": np.ascontiguousarray(bq_c),
            "cosT": cosT,
            "sinT": sinT,
            "woutT": woutT,
            "bout": bo_sb,
        })
    return in_maps


def kernel_run(inputs, trace=False, tmpdir=None):
    nc = _get_nc()
    in_maps = _host_prep(**inputs)
    res = run_bass_kernel_spmd(
        nc, in_maps, list(range(NC)), trace=trace, tmpdir=tmpdir)
    outT = np.concatenate(
        [np.asarray(res.results[c]["out"], dtype=np.float32) for c in range(NC)],
        axis=1)                                             # [1024, 4096]
    out = np.ascontiguousarray(outT.T).reshape(B, L, D)
    return out, res


def kernel(**inputs):
    out, _ = kernel_run(inputs, trace=False)
    return out


# revision 31
# speedup vs baseline: 1.1679x; 1.0958x over previous
"""Distributed multi-head attention (QKV proj + RoPE + softmax attention + out proj)
on 8 TRN2 NeuronCores.

Sharding: tensor-parallel over heads. Core c owns heads (2c, 2c+1):
  - qkv^T = W_c @ x^T for its 384 channels over all 4096 tokens (bf16 matmul)
  - RoPE on q,k (fp32, partition-swap via SBUF-SBUF DMA)
  - scores^T = k @ q^T per (batch, head): both heads' scores go into one
    2-bank PSUM tile (row-tiled matmuls), one exp [128,1024] on ScalarE
  - ctx^T = [v | 1] @ expS^T : M=65 matmul computes context + softmax denominator
    (ones column baked into the transposed-v layout)
  - per-(qt,head) pipelined normalization: approx reciprocal + partition-
    broadcast via a stride-0 DRAM read; batch-1 QKV/rope work is drip-fed
    into batch-0's ACT-bound attention to keep the TensorEngine warm;
    dummy matmuls bridge the AllToAll window so stage 4 runs at 2.4 GHz
  - AllToAll redistributes ctx: head-sharded -> token-sharded (512 tok/core)
  - out^T = W_out^T.T @ ctx_full^T + b_out for the core's 512 tokens

Host side: transposes/shards weights, runs SPMD, gathers [1024, 512] fp32 per core,
transposes to [2, 2048, 1024].
"""

import numpy as np
import ml_dtypes

import concourse.bass as bass
import concourse.tile as tile
from concourse import bacc, mybir
from concourse.bass_utils import run_bass_kernel_spmd
from concourse.masks import make_identity

BF16 = ml_dtypes.bfloat16

B, L, D, H, Hd = 2, 2048, 1024, 16, 64
T = B * L              # 4096 tokens
NC = 8                 # cores
HPC = H // NC          # 2 heads per core
TOK = T // NC          # 512 token shard per core
NT = T // 512          # 8 token n-tiles of 512
KT = L // 128          # 16 k-tiles per batch
QT = L // 512          # 4 q-tiles per batch

F32 = mybir.dt.float32
BF = mybir.dt.bfloat16


def build(debug=False):
    nc = bacc.Bacc(None, target_bir_lowering=False, num_devices=NC)

    xT = nc.dram_tensor("xT", [D, T], BF, kind="ExternalInput")          # x^T, replicated
    wq = nc.dram_tensor("wqkvT", [D, 3 * 128], BF, kind="ExternalInput")  # W_c^T per core
    bq = nc.dram_tensor("bqkv", [128, 3], F32, kind="ExternalInput")      # bias cols q,k,v
    cosT = nc.dram_tensor("cosT", [128, L], F32, kind="ExternalInput")
    sinT = nc.dram_tensor("sinT", [128, L], F32, kind="ExternalInput")    # sign-folded sin
    wo = nc.dram_tensor("woutT", [D, D], BF, kind="ExternalInput")        # W_out^T, replicated
    bo = nc.dram_tensor("bout", [128, NC], F32, kind="ExternalInput")
    out = nc.dram_tensor("out", [D, TOK], F32, kind="ExternalOutput")

    with tile.TileContext(nc) as tc:
        with tc.tile_pool(name="const", bufs=1) as const, \
             tc.tile_pool(name="big", bufs=1) as big, \
             tc.tile_pool(name="rope", bufs=3) as rope, \
             tc.tile_pool(name="es", bufs=8) as esp, \
             tc.tile_pool(name="cu", bufs=10) as cup, \
             tc.tile_pool(name="small", bufs=3) as small, \
             tc.tile_pool(name="psum", bufs=1, space="PSUM") as psum, \
             tc.tile_pool(name="dram", bufs=1, space="DRAM") as dram:

            # ---------------- constants / weights (loaded before x!) ----------
            ident = const.tile([128, 128], BF, tag="ident")
            make_identity(nc, ident[:])
            ones_bc = const.tile([1, 64], F32, tag="ones_bc")
            nc.vector.memset(ones_bc[:], 1.0)

            bq_sb = const.tile([128, 3], F32, tag="bq")
            nc.sync.dma_start(bq_sb[:], bq[:])
            bo_sb = const.tile([128, NC], F32, tag="bo")
            cos_sb = const.tile([128, L], F32, tag="cos")
            nc.sync.dma_start(cos_sb[:], cosT[:])
            sin_sb = const.tile([128, L], F32, tag="sin")
            nc.sync.dma_start(sin_sb[:], sinT[:])
            w_sb = []
            for k in range(8):
                t = big.tile([128, 3 * 128], BF, tag=f"w{k}", name=f"w{k}")
                nc.sync.dma_start(t[:], wq[128 * k:128 * (k + 1), :])
                w_sb.append(t)
            wo_sb = [big.tile([128, D], BF, tag=f"wo{k}", name=f"wo_{k}")
                     for k in range(8)]


            qT_sb = big.tile([128, T], BF, tag="qT")
            kT_sb = big.tile([128, T], BF, tag="kT")
            v_sb = big.tile([128, T], BF, tag="v")
            # transposed v with a built-in ones column: [tok%128, blk, head, 65]
            vn_sb = big.tile([128, T // 128, HPC, 65], BF, tag="vn")
            nc.vector.memset(vn_sb[:, :, :, 64:65], 1.0)

            a2a_in = dram.tile([NC, 128, TOK], BF, tag="a2a_in")
            a2a_out = dram.tile([NC, 128, TOK], BF, tag="a2a_out")

            # ---------------- per-stage emitters ------------------------------
            _xc_cache = {}

            def stage1_load(n):
                ts = slice(512 * n, 512 * (n + 1))
                xc = []
                for k in range(8):
                    t = rope.tile([128, 512], BF, tag="xc", bufs=24,
                                  name=f"xc_{n}_{k}")
                    nc.sync.dma_start(t[:], xT[128 * k:128 * (k + 1), ts])
                    xc.append(t)
                _xc_cache[n] = xc

            def stage1_qkv_m(n, m):
                """QKV matmul + bias (+rope for q/k) for one (n-tile, m)."""
                ts = slice(512 * n, 512 * (n + 1))
                cs = slice(512 * (n % QT), 512 * (n % QT) + 512)
                xc = _xc_cache[n]
                if True:
                    ps = psum.tile([128, 512], F32, tag="st", bufs=3,
                                   name=f"s1_{n}_{m}")
                    for k in range(8):
                        nc.tensor.matmul(
                            ps[:],
                            w_sb[k][:, 128 * m:128 * (m + 1)],
                            xc[k][:],
                            start=(k == 0), stop=(k == 7),
                        )
                    if m < 2:  # q or k: ACT evicts (+bias) fast to free the
                        # PSUM slot; rope split across DVE and GpSimd
                        dst = qT_sb if m == 0 else kT_sb
                        qb = rope.tile([128, 512], F32, tag="qb", bufs=6,
                                       name=f"qb_{n}_{m}")
                        nc.scalar.activation(
                            qb[:], ps[:],
                            mybir.ActivationFunctionType.Identity,
                            bias=bq_sb[:, m:m + 1])
                        qc = rope.tile([128, 512], F32, tag="qc", name=f"qc_{n}_{m}")
                        nc.vector.tensor_tensor(
                            qc[:], qb[:], cos_sb[:, cs], mybir.AluOpType.mult)
                        qs = rope.tile([128, 512], F32, tag="qs", name=f"qs_{n}_{m}")
                        nc.gpsimd.tensor_tensor(
                            qs[:], qb[:], sin_sb[:, cs], mybir.AluOpType.mult)
                        qw = rope.tile([128, 512], F32, tag="qw", name=f"qw_{n}_{m}")
                        for blk in range(4):
                            src = 32 * (blk ^ 1)
                            nc.gpsimd.dma_start(
                                qw[32 * blk:32 * blk + 32, :],
                                qs[src:src + 32, :])
                        nc.vector.tensor_tensor(
                            dst[:, ts], qc[:], qw[:], mybir.AluOpType.add)
                    else:  # v: bias only, straight to bf16
                        nc.scalar.activation(
                            v_sb[:, ts], ps[:],
                            mybir.ActivationFunctionType.Identity,
                            bias=bq_sb[:, 2:3])

            def stage1_qkv(n):
                stage1_load(n)
                for m in range(3):
                    stage1_qkv_m(n, m)

            def stage1_vtr(j):
                """Transpose one 128-token block of v into vn (both heads)."""
                tp = psum.tile([128, 128], BF, tag="st", bufs=3, name=f"tr_{j}")
                nc.tensor.transpose(tp[:], v_sb[:, 128 * j:128 * (j + 1)], ident[:])
                for h in range(HPC):
                    nc.vector.tensor_copy(
                        vn_sb[:, j, h, 0:64], tp[:, 64 * h:64 * (h + 1)])

            def stage2_open(b, qt):
                return [psum.tile([65, 512], F32, tag=f"ctx{h}", bufs=1,
                                  name=f"ctx_{b}_{qt}_{h}")
                        for h in range(HPC)]

            def stage2_kts(b, qt, ctxs, kts, fill_iter):
                qsl = slice(2048 * b + 512 * qt, 2048 * b + 512 * qt + 512)
                for kt in kts:
                    ksl = slice(2048 * b + 128 * kt, 2048 * b + 128 * kt + 128)
                    blk = 16 * b + kt
                    st2 = psum.tile([128, 1024], F32, tag="st", bufs=3,
                                    name=f"st_{b}_{qt}_{kt}")
                    for h in range(HPC):
                        nc.tensor.matmul(
                            st2[:, 512 * h:512 * (h + 1)],
                            kT_sb[64 * h:64 * (h + 1), ksl],
                            qT_sb[64 * h:64 * (h + 1), qsl],
                            start=True, stop=True)
                    es = esp.tile([128, 1024], BF, tag="es",
                                  name=f"es_{b}_{qt}_{kt}")
                    nc.scalar.activation(
                        es[:], st2[:], mybir.ActivationFunctionType.Exp)
                    for h in range(HPC):
                        nc.tensor.matmul(
                            ctxs[h][:],
                            vn_sb[:, blk, h, :],
                            es[:, 512 * h:512 * (h + 1)],
                            start=(kt == 0), stop=(kt == KT - 1))
                    fill_iter(b, qt, kt)

            def stage2_qtile(b, qt, ctx_evict, fill_iter):
                ctxs = stage2_open(b, qt)
                stage2_kts(b, qt, ctxs, range(KT), fill_iter)
                ctx_evict(qt, ctxs)

            def run_batch(b, fill_iter, qts=range(QT), pre_ctxs=None):
                """Stage-2 for one batch; per-(qt,h) pipelined normalization."""

                def ctx_evict(qt, ctxs):
                    for h in range(HPC):
                        cu = cup.tile([65, 512], F32, tag="cu",
                                      name=f"cu_{b}_{qt}_{h}")
                        nc.vector.tensor_copy(cu[:], ctxs[h][:])
                        dn = small.tile([1, 512], F32, tag="dn",
                                        name=f"dn_{b}_{qt}_{h}", bufs=3)
                        nc.vector.tensor_copy(dn[:], cu[64:65, :])
                        rc = small.tile([1, 512], F32, tag="rc",
                                        name=f"rc_{b}_{qt}_{h}", bufs=3)
                        nc.vector.reciprocal_approx_fast(rc[:], dn[:])
                        dr = dram.tile([1, 512], F32, tag="dr",
                                       name=f"dr_{b}_{qt}_{h}", bufs=4)
                        nc.gpsimd.dma_start(dr[:], rc[:])
                        bca = small.tile([64, 512], F32, tag="bca",
                                         name=f"bca_{b}_{qt}_{h}", bufs=3)
                        dr_ap = dr[:]
                        bcast_src = bass.AP(
                            tensor=dr_ap.tensor, offset=dr_ap.offset,
                            ap=[[0, 32]] + [list(p) for p in dr_ap.ap])
                        nc.gpsimd.dma_start(bca[0:32, :], bcast_src)
                        nc.sync.dma_start(bca[32:64, :], bcast_src)
                        cn = small.tile([64, 512], BF, tag="cn",
                                        name=f"cn_{b}_{qt}_{h}")
                        nc.vector.tensor_tensor(
                            cn[:], cu[0:64, :], bca[:],
                            mybir.AluOpType.mult)
                        nc.sync.dma_start(
                            a2a_in[QT * b + qt, 64 * h:64 * (h + 1), :], cn[:])

                if pre_ctxs is not None:
                    ctx_evict(0, pre_ctxs)
                for qt in qts:
                    stage2_qtile(b, qt, ctx_evict, fill_iter)
                return ctx_evict

            # ---------------- emission schedule -------------------------------
            # stage 1 for batch 0 (transposes follow each n-tile's v)
            for n in range(QT):
                stage1_qkv(n)
                for j in range(4 * n, 4 * n + 4):
                    stage1_vtr(j)

            # stage 2 for batch 0, with stage-1(b=1) units drip-fed to keep PE busy
            b1_units = []
            for n in range(QT, NT):
                b1_units.append(lambda n=n: stage1_load(n))
                for m in range(3):
                    b1_units.append(lambda n=n, m=m: stage1_qkv_m(n, m))
                for j in range(4 * n, 4 * n + 4):
                    b1_units.append(lambda j=j: stage1_vtr(j))
            unit_idx = [0]
            count = [0]
            # 64 kt-iterations in batch 0; 32 fill units -> every 2nd iteration
            def fill_iter(b, qt, kt):
                count[0] += 1
                if b == 0 and count[0] % 2 == 0 and unit_idx[0] < len(b1_units):
                    b1_units[unit_idx[0]]()
                    unit_idx[0] += 1

            run_batch(0, fill_iter)
            for k in range(8):
                nc.sync.dma_start(wo_sb[k][:], wo[128 * k:128 * (k + 1), :])
            nc.sync.dma_start(bo_sb[:], bo[:])
            while unit_idx[0] < len(b1_units):
                b1_units[unit_idx[0]]()
                unit_idx[0] += 1
            # dummy matmuls keep the PE HAM-warm through ACT-bound b=1
            dummy_scr = small.tile([1, 512], F32, tag="dscr", name="dscr", bufs=1)
            dummy_cnt = [0]

            def emit_dummies(n):
                for _ in range(n):
                    i = dummy_cnt[0]
                    dummy_cnt[0] += 1
                    if i % 8 == 0:
                        dummy_cnt.append(psum.tile(
                            [128, 512], F32, tag="st", bufs=3,
                            name=f"dmy{i}"))
                    dp = dummy_cnt[-1]
                    nc.tensor.matmul(
                        dp[:], ident[:], kT_sb[:, 0:512],
                        start=(i % 8 == 0), stop=(i % 8 == 7))
                    if i % 8 == 7:
                        nc.vector.tensor_copy(
                            dummy_scr[:, 2 * (i // 8):2 * (i // 8) + 2],
                            dp[0:1, 0:2])

            run_batch(1, lambda b, qt, kt: None)

            # ---------------- stage 3: AllToAll ------------------------------
            nc.gpsimd.collective_compute(
                "AllToAll",
                mybir.AluOpType.bypass,
                replica_groups=[list(range(NC))],
                ins=[a2a_in.opt()],
                outs=[a2a_out.opt()],
            )

            emit_dummies(120)
            dscr_dr = dram.tile([1, 512], F32, tag="dscr_dr", name="dscr_dr")
            nc.sync.dma_start(dscr_dr[:], dummy_scr[:])

            # ---------------- stage 4: out projection ------------------------
            ctxf_sb = []
            for k in range(8):
                t = big.tile([128, TOK], BF, tag=f"cf{k}", name=f"cf{k}")
                nc.sync.dma_start(t[:], a2a_out[k, :, :])
                ctxf_sb.append(t)
            for m in range(8):
                pso = psum.tile([128, TOK], F32, tag="st", bufs=3, name=f"o_{m}")
                for k in range(8):
                    nc.tensor.matmul(
                        pso[:],
                        wo_sb[k][:, 128 * m:128 * (m + 1)],
                        ctxf_sb[k][:],
                        start=(k == 0), stop=(k == 7))
                os_t = small.tile([128, TOK], F32, tag="os", name=f"os_{m}")
                nc.scalar.activation(
                    os_t[:], pso[:],
                    mybir.ActivationFunctionType.Identity,
                    bias=bo_sb[:, m:m + 1])
                (nc.sync if m % 2 == 0 else nc.gpsimd).dma_start(
                    out[128 * m:128 * (m + 1), :], os_t[:])

    nc.compile()
    return nc


_NC_CACHE = None


def _get_nc():
    global _NC_CACHE
    if _NC_CACHE is None:
        _NC_CACHE = build()
    return _NC_CACHE


def _host_prep(x, W_qkv, b_qkv, W_out, b_out):
    x = np.asarray(x, dtype=np.float32)
    W_qkv = np.asarray(W_qkv, dtype=np.float32)
    b_qkv = np.asarray(b_qkv, dtype=np.float32)
    W_out = np.asarray(W_out, dtype=np.float32)
    b_out = np.asarray(b_out, dtype=np.float32)

    scale = 1.0 / np.sqrt(Hd)
    xT = np.ascontiguousarray(x.reshape(T, D).T).astype(BF16)

    # rope tables (token position within batch), channel-transposed + sign-folded
    inv_freq = 1.0 / (10000.0 ** (np.arange(0, Hd, 2, dtype=np.float32) / Hd))  # [32]
    t_pos = np.arange(L, dtype=np.float32)
    freqs = np.outer(t_pos, inv_freq)                       # [L, 32]
    emb = np.concatenate([freqs, freqs], axis=1)            # [L, 64]
    cos_t = np.cos(emb).T.astype(np.float32)                # [64, L]
    sin_t = np.sin(emb).T.astype(np.float32)                # [64, L]
    sin2 = sin_t.copy()
    sin2[32:, :] *= -1.0                                    # s''[d] = +sin d<32, -sin d>=32
    cosT = np.ascontiguousarray(np.tile(cos_t, (2, 1)))     # [128, L]
    sinT = np.ascontiguousarray(np.tile(sin2, (2, 1)))

    woutT = np.ascontiguousarray(W_out.T).astype(BF16)      # [D, D]
    bo_sb = np.ascontiguousarray(b_out.reshape(NC, 128).T)  # [128, 8]

    in_maps = []
    for c in range(NC):
        r = slice(128 * c, 128 * (c + 1))
        Wq = W_qkv[0 * D:1 * D][r] * scale
        Wk = W_qkv[1 * D:2 * D][r]
        Wv = W_qkv[2 * D:3 * D][r]
        Wc = np.concatenate([Wq, Wk, Wv], axis=0)           # [384, 1024]
        WcT = np.ascontiguousarray(Wc.T).astype(BF16)       # [1024, 384]
        bq_c = np.stack([
            b_qkv[0 * D:1 * D][r] * scale,
            b_qkv[1 * D:2 * D][r],
            b_qkv[2 * D:3 * D][r],
        ], axis=1).astype(np.float32)                       # [128, 3]
        in_maps.append({
            "xT": xT,
            "wqkvT": WcT,
            "bqkv": np.ascontiguousarray(bq_c),
            "cosT": cosT,
            "sinT": sinT,
            "woutT": woutT,
            "bout": bo_sb,
        })
    return in_maps


def kernel_run(inputs, trace=False, tmpdir=None):
    nc = _get_nc()
    in_maps = _host_prep(**inputs)
    res = run_bass_kernel_spmd(
        nc, in_maps, list(range(NC)), trace=trace, tmpdir=tmpdir)
    outT = np.concatenate(
        [np.asarray(res.results[c]["out"], dtype=np.float32) for c in range(NC)],
        axis=1)                                             # [1024, 4096]
    out = np.ascontiguousarray(outT.T).reshape(B, L, D)
    return out, res


def kernel(**inputs):
    out, _ = kernel_run(inputs, trace=False)
    return out


# revision 32
# speedup vs baseline: 1.2725x; 1.0895x over previous
"""Distributed multi-head attention (QKV proj + RoPE + softmax attention + out proj)
on 8 TRN2 NeuronCores.

Sharding: tensor-parallel over heads. Core c owns heads (2c, 2c+1):
  - qkv^T = W_c @ x^T for its 384 channels over all 4096 tokens (bf16 matmul)
  - RoPE on q,k (fp32, partition-swap via SBUF-SBUF DMA)
  - scores^T = k @ q^T per (batch, head): both heads' scores go into one
    2-bank PSUM tile (row-tiled matmuls), one exp [128,1024] on ScalarE
  - ctx^T = [v | 1] @ expS^T : M=65 matmul computes context + softmax denominator
    (ones column baked into the transposed-v layout)
  - per-(qt,head) pipelined normalization: approx reciprocal + partition-
    broadcast via a stride-0 DRAM read; batch-1 QKV/rope work is drip-fed
    into batch-0's ACT-bound attention to keep the TensorEngine warm;
    dummy matmuls bridge the AllToAll window so stage 4 runs at 2.4 GHz
  - AllToAll redistributes ctx: head-sharded -> token-sharded (512 tok/core)
  - out^T = W_out^T.T @ ctx_full^T + b_out for the core's 512 tokens

Host side: transposes/shards weights, runs SPMD, gathers [1024, 512] fp32 per core,
transposes to [2, 2048, 1024].
"""

import numpy as np
import ml_dtypes

import concourse.bass as bass
import concourse.tile as tile
from concourse import bacc, mybir
from concourse.bass_utils import run_bass_kernel_spmd
from concourse.masks import make_identity

BF16 = ml_dtypes.bfloat16

B, L, D, H, Hd = 2, 2048, 1024, 16, 64
T = B * L              # 4096 tokens
NC = 8                 # cores
HPC = H // NC          # 2 heads per core
TOK = T // NC          # 512 token shard per core
NT = T // 512          # 8 token n-tiles of 512
KT = L // 128          # 16 k-tiles per batch
QT = L // 512          # 4 q-tiles per batch

F32 = mybir.dt.float32
BF = mybir.dt.bfloat16


def build(debug=False):
    nc = bacc.Bacc(None, target_bir_lowering=False, num_devices=NC)

    xT = nc.dram_tensor("xT", [D, T], BF, kind="ExternalInput")          # x^T, replicated
    wq = nc.dram_tensor("wqkvT", [D, 3 * 128], BF, kind="ExternalInput")  # W_c^T per core
    bq = nc.dram_tensor("bqkv", [128, 3], F32, kind="ExternalInput")      # bias cols q,k,v
    cosT = nc.dram_tensor("cosT", [128, L], F32, kind="ExternalInput")
    sinT = nc.dram_tensor("sinT", [128, L], F32, kind="ExternalInput")    # sign-folded sin
    wo = nc.dram_tensor("woutT", [D, D], BF, kind="ExternalInput")        # W_out^T, replicated
    bo = nc.dram_tensor("bout", [128, NC], F32, kind="ExternalInput")
    out = nc.dram_tensor("out", [D, TOK], F32, kind="ExternalOutput")

    with tile.TileContext(nc) as tc:
        with tc.tile_pool(name="const", bufs=1) as const, \
             tc.tile_pool(name="big", bufs=1) as big, \
             tc.tile_pool(name="rope", bufs=3) as rope, \
             tc.tile_pool(name="es", bufs=10) as esp, \
             tc.tile_pool(name="cu", bufs=12) as cup, \
             tc.tile_pool(name="small", bufs=3) as small, \
             tc.tile_pool(name="psum", bufs=1, space="PSUM") as psum, \
             tc.tile_pool(name="dram", bufs=1, space="DRAM") as dram:

            # ---------------- constants / weights (loaded before x!) ----------
            ident = const.tile([128, 128], BF, tag="ident")
            make_identity(nc, ident[:])
            ones_bc = const.tile([1, 64], F32, tag="ones_bc")
            nc.vector.memset(ones_bc[:], 1.0)

            bq_sb = const.tile([128, 3], F32, tag="bq")
            nc.sync.dma_start(bq_sb[:], bq[:])
            bo_sb = const.tile([128, NC], F32, tag="bo")
            cos_sb = const.tile([128, L], F32, tag="cos")
            nc.sync.dma_start(cos_sb[:], cosT[:])
            sin_sb = const.tile([128, L], F32, tag="sin")
            nc.sync.dma_start(sin_sb[:], sinT[:])
            w_sb = []
            for k in range(8):
                t = big.tile([128, 3 * 128], BF, tag=f"w{k}", name=f"w{k}")
                nc.sync.dma_start(t[:], wq[128 * k:128 * (k + 1), :])
                w_sb.append(t)
            wo_sb = [big.tile([128, D], BF, tag=f"wo{k}", name=f"wo_{k}")
                     for k in range(8)]


            qT_sb = big.tile([128, T], BF, tag="qT")
            kT_sb = big.tile([128, T], BF, tag="kT")
            v_sb = big.tile([128, T], BF, tag="v")
            # transposed v with a built-in ones column: [tok%128, blk, head, 65]
            vn_sb = big.tile([128, T // 128, HPC, 65], BF, tag="vn")
            nc.vector.memset(vn_sb[:, :, :, 64:65], 1.0)

            a2a_in = dram.tile([NC, 128, TOK], BF, tag="a2a_in")
            a2a_out = dram.tile([NC, 128, TOK], BF, tag="a2a_out")

            # ---------------- per-stage emitters ------------------------------
            _xc_cache = {}

            def stage1_load(n):
                ts = slice(512 * n, 512 * (n + 1))
                xc = []
                for k in range(8):
                    t = rope.tile([128, 512], BF, tag="xc", bufs=24,
                                  name=f"xc_{n}_{k}")
                    nc.sync.dma_start(t[:], xT[128 * k:128 * (k + 1), ts])
                    xc.append(t)
                _xc_cache[n] = xc

            def stage1_qkv_m(n, m):
                """QKV matmul + bias (+rope for q/k) for one (n-tile, m)."""
                ts = slice(512 * n, 512 * (n + 1))
                cs = slice(512 * (n % QT), 512 * (n % QT) + 512)
                xc = _xc_cache[n]
                if True:
                    ps = psum.tile([128, 512], F32, tag="st", bufs=3,
                                   name=f"s1_{n}_{m}")
                    for k in range(8):
                        nc.tensor.matmul(
                            ps[:],
                            w_sb[k][:, 128 * m:128 * (m + 1)],
                            xc[k][:],
                            start=(k == 0), stop=(k == 7),
                        )
                    if m < 2:  # q or k: ACT evicts (+bias) fast to free the
                        # PSUM slot; rope split across DVE and GpSimd
                        dst = qT_sb if m == 0 else kT_sb
                        qb = rope.tile([128, 512], F32, tag="qb", bufs=6,
                                       name=f"qb_{n}_{m}")
                        nc.scalar.activation(
                            qb[:], ps[:],
                            mybir.ActivationFunctionType.Identity,
                            bias=bq_sb[:, m:m + 1])
                        qc = rope.tile([128, 512], F32, tag="qc", name=f"qc_{n}_{m}")
                        nc.vector.tensor_tensor(
                            qc[:], qb[:], cos_sb[:, cs], mybir.AluOpType.mult)
                        qs = rope.tile([128, 512], F32, tag="qs", name=f"qs_{n}_{m}")
                        nc.gpsimd.tensor_tensor(
                            qs[:], qb[:], sin_sb[:, cs], mybir.AluOpType.mult)
                        qw = rope.tile([128, 512], F32, tag="qw", name=f"qw_{n}_{m}")
                        for blk in range(4):
                            src = 32 * (blk ^ 1)
                            nc.gpsimd.dma_start(
                                qw[32 * blk:32 * blk + 32, :],
                                qs[src:src + 32, :])
                        nc.vector.tensor_tensor(
                            dst[:, ts], qc[:], qw[:], mybir.AluOpType.add)
                    else:  # v: bias only, straight to bf16
                        nc.scalar.activation(
                            v_sb[:, ts], ps[:],
                            mybir.ActivationFunctionType.Identity,
                            bias=bq_sb[:, 2:3])

            def stage1_qkv(n):
                stage1_load(n)
                for m in range(3):
                    stage1_qkv_m(n, m)

            def stage1_vtr(j):
                """Transpose one 128-token block of v into vn (both heads)."""
                tp = psum.tile([128, 128], BF, tag="st", bufs=3, name=f"tr_{j}")
                nc.tensor.transpose(tp[:], v_sb[:, 128 * j:128 * (j + 1)], ident[:])
                for h in range(HPC):
                    nc.vector.tensor_copy(
                        vn_sb[:, j, h, 0:64], tp[:, 64 * h:64 * (h + 1)])

            def stage2_open(b, qt):
                return [psum.tile([65, 512], F32, tag=f"ctx{h}", bufs=1,
                                  name=f"ctx_{b}_{qt}_{h}")
                        for h in range(HPC)]

            def stage2_kts(b, qt, ctxs, kts, fill_iter):
                qsl = slice(2048 * b + 512 * qt, 2048 * b + 512 * qt + 512)
                for kt in kts:
                    ksl = slice(2048 * b + 128 * kt, 2048 * b + 128 * kt + 128)
                    blk = 16 * b + kt
                    st2 = psum.tile([128, 1024], F32, tag="st", bufs=3,
                                    name=f"st_{b}_{qt}_{kt}")
                    for h in range(HPC):
                        nc.tensor.matmul(
                            st2[:, 512 * h:512 * (h + 1)],
                            kT_sb[64 * h:64 * (h + 1), ksl],
                            qT_sb[64 * h:64 * (h + 1), qsl],
                            start=True, stop=True)
                    es = esp.tile([128, 1024], BF, tag="es",
                                  name=f"es_{b}_{qt}_{kt}")
                    nc.scalar.activation(
                        es[:], st2[:], mybir.ActivationFunctionType.Exp)
                    for h in range(HPC):
                        nc.tensor.matmul(
                            ctxs[h][:],
                            vn_sb[:, blk, h, :],
                            es[:, 512 * h:512 * (h + 1)],
                            start=(kt == 0), stop=(kt == KT - 1))
                    fill_iter(b, qt, kt)

            def stage2_qtile(b, qt, ctx_evict, fill_iter):
                ctxs = stage2_open(b, qt)
                stage2_kts(b, qt, ctxs, range(KT), fill_iter)
                ctx_evict(qt, ctxs)

            def run_batch(b, fill_iter, qts=range(QT), pre_ctxs=None):
                """Stage-2 for one batch; per-(qt,h) pipelined normalization."""

                def ctx_evict(qt, ctxs):
                    for h in range(HPC):
                        cu = cup.tile([65, 512], F32, tag="cu",
                                      name=f"cu_{b}_{qt}_{h}")
                        nc.vector.tensor_copy(cu[:], ctxs[h][:])
                        dn = small.tile([1, 512], F32, tag="dn",
                                        name=f"dn_{b}_{qt}_{h}", bufs=3)
                        nc.vector.tensor_copy(dn[:], cu[64:65, :])
                        rc = small.tile([1, 512], F32, tag="rc",
                                        name=f"rc_{b}_{qt}_{h}", bufs=3)
                        nc.vector.reciprocal_approx_fast(rc[:], dn[:])
                        dr = dram.tile([1, 512], F32, tag="dr",
                                       name=f"dr_{b}_{qt}_{h}", bufs=4)
                        nc.gpsimd.dma_start(dr[:], rc[:])
                        bca = small.tile([64, 512], F32, tag="bca",
                                         name=f"bca_{b}_{qt}_{h}", bufs=3)
                        dr_ap = dr[:]
                        bcast_src = bass.AP(
                            tensor=dr_ap.tensor, offset=dr_ap.offset,
                            ap=[[0, 32]] + [list(p) for p in dr_ap.ap])
                        nc.gpsimd.dma_start(bca[0:32, :], bcast_src)
                        nc.sync.dma_start(bca[32:64, :], bcast_src)
                        cn = small.tile([64, 512], BF, tag="cn",
                                        name=f"cn_{b}_{qt}_{h}")
                        nc.vector.tensor_tensor(
                            cn[:], cu[0:64, :], bca[:],
                            mybir.AluOpType.mult)
                        nc.sync.dma_start(
                            a2a_in[QT * b + qt, 64 * h:64 * (h + 1), :], cn[:])

                if pre_ctxs is not None:
                    ctx_evict(0, pre_ctxs)
                for qt in qts:
                    stage2_qtile(b, qt, ctx_evict, fill_iter)
                return ctx_evict

            # ---------------- emission schedule -------------------------------
            # stage 1 for batch 0 (transposes follow each n-tile's v)
            for n in range(QT):
                stage1_qkv(n)
                for j in range(4 * n, 4 * n + 4):
                    stage1_vtr(j)

            # stage 2 for batch 0, with stage-1(b=1) units drip-fed to keep PE busy
            b1_units = []
            for n in range(QT, NT):
                b1_units.append(lambda n=n: stage1_load(n))
                for m in range(3):
                    b1_units.append(lambda n=n, m=m: stage1_qkv_m(n, m))
                for j in range(4 * n, 4 * n + 4):
                    b1_units.append(lambda j=j: stage1_vtr(j))
            unit_idx = [0]
            count = [0]
            # 64 kt-iterations in batch 0; 32 fill units -> every 2nd iteration
            def fill_iter(b, qt, kt):
                count[0] += 1
                if b == 0 and count[0] % 2 == 0 and unit_idx[0] < len(b1_units):
                    b1_units[unit_idx[0]]()
                    unit_idx[0] += 1

            run_batch(0, fill_iter)
            for k in range(8):
                nc.sync.dma_start(wo_sb[k][:], wo[128 * k:128 * (k + 1), :])
            nc.sync.dma_start(bo_sb[:], bo[:])
            while unit_idx[0] < len(b1_units):
                b1_units[unit_idx[0]]()
                unit_idx[0] += 1
            # dummy matmuls keep the PE HAM-warm through ACT-bound b=1
            dummy_scr = small.tile([1, 512], F32, tag="dscr", name="dscr", bufs=1)
            dummy_cnt = [0]

            def emit_dummies(n):
                for _ in range(n):
                    i = dummy_cnt[0]
                    dummy_cnt[0] += 1
                    if i % 8 == 0:
                        dummy_cnt.append(psum.tile(
                            [128, 512], F32, tag="st", bufs=3,
                            name=f"dmy{i}"))
                    dp = dummy_cnt[-1]
                    nc.tensor.matmul(
                        dp[:], ident[:], kT_sb[:, 0:512],
                        start=(i % 8 == 0), stop=(i % 8 == 7))
                    if i % 8 == 7:
                        nc.vector.tensor_copy(
                            dummy_scr[:, 2 * (i // 8):2 * (i // 8) + 2],
                            dp[0:1, 0:2])

            run_batch(1, lambda b, qt, kt: None)

            # ---------------- stage 3: AllToAll ------------------------------
            nc.gpsimd.collective_compute(
                "AllToAll",
                mybir.AluOpType.bypass,
                replica_groups=[list(range(NC))],
                ins=[a2a_in.opt()],
                outs=[a2a_out.opt()],
            )

            emit_dummies(170)
            dscr_dr = dram.tile([1, 512], F32, tag="dscr_dr", name="dscr_dr")
            nc.sync.dma_start(dscr_dr[:], dummy_scr[:])

            # ---------------- stage 4: out projection ------------------------
            ctxf_sb = []
            for k in range(8):
                t = big.tile([128, TOK], BF, tag=f"cf{k}", name=f"cf{k}")
                (nc.sync if k % 2 == 0 else nc.gpsimd).dma_start(
                    t[:], a2a_out[k, :, :])
                ctxf_sb.append(t)
            for m in range(8):
                pso = psum.tile([128, TOK], F32, tag="st", bufs=3, name=f"o_{m}")
                for k in range(8):
                    nc.tensor.matmul(
                        pso[:],
                        wo_sb[k][:, 128 * m:128 * (m + 1)],
                        ctxf_sb[k][:],
                        start=(k == 0), stop=(k == 7))
                os_t = small.tile([128, TOK], F32, tag="os", name=f"os_{m}")
                nc.scalar.activation(
                    os_t[:], pso[:],
                    mybir.ActivationFunctionType.Identity,
                    bias=bo_sb[:, m:m + 1])
                nc.sync.dma_start(
                    out[128 * m:128 * (m + 1), 0:256], os_t[:, 0:256])
                nc.gpsimd.dma_start(
                    out[128 * m:128 * (m + 1), 256:512], os_t[:, 256:512])

    nc.compile()
    return nc


_NC_CACHE = None


def _get_nc():
    global _NC_CACHE
    if _NC_CACHE is None:
        _NC_CACHE = build()
    return _NC_CACHE


def _host_prep(x, W_qkv, b_qkv, W_out, b_out):
    x = np.asarray(x, dtype=np.float32)
    W_qkv = np.asarray(W_qkv, dtype=np.float32)
    b_qkv = np.asarray(b_qkv, dtype=np.float32)
    W_out = np.asarray(W_out, dtype=np.float32)
    b_out = np.asarray(b_out, dtype=np.float32)

    scale = 1.0 / np.sqrt(Hd)
    xT = np.ascontiguousarray(x.reshape(T, D).T).astype(BF16)

    # rope tables (token position within batch), channel-transposed + sign-folded
    inv_freq = 1.0 / (10000.0 ** (np.arange(0, Hd, 2, dtype=np.float32) / Hd))  # [32]
    t_pos = np.arange(L, dtype=np.float32)
    freqs = np.outer(t_pos, inv_freq)                       # [L, 32]
    emb = np.concatenate([freqs, freqs], axis=1)            # [L, 64]
    cos_t = np.cos(emb).T.astype(np.float32)                # [64, L]
    sin_t = np.sin(emb).T.astype(np.float32)                # [64, L]
    sin2 = sin_t.copy()
    sin2[32:, :] *= -1.0                                    # s''[d] = +sin d<32, -sin d>=32
    cosT = np.ascontiguousarray(np.tile(cos_t, (2, 1)))     # [128, L]
    sinT = np.ascontiguousarray(np.tile(sin2, (2, 1)))

    woutT = np.ascontiguousarray(W_out.T).astype(BF16)      # [D, D]
    bo_sb = np.ascontiguousarray(b_out.reshape(NC, 128).T)  # [128, 8]

    in_maps = []
    for c in range(NC):
        r = slice(128 * c, 128 * (c + 1))
        Wq = W_qkv[0 * D:1 * D][r] * scale
        Wk = W_qkv[1 * D:2 * D][r]
        Wv = W_qkv[2 * D:3 * D][r]
        Wc = np.concatenate([Wq, Wk, Wv], axis=0)           # [384, 1024]
        WcT = np.ascontiguousarray(Wc.T).astype(BF16)       # [1024, 384]
        bq_c = np.stack([
            b_qkv[0 * D:1 * D][r] * scale,
            b_qkv[1 * D:2 * D][r],
            b_qkv[2 * D:3 * D][r],
        ], axis=1).astype(np.float32)                       # [128, 3]
        in_maps.append({
            "xT": xT,
            "wqkvT": WcT,
            "bqkv": np.ascontiguousarray(bq_c),
            "cosT": cosT,
            "sinT": sinT,
            "woutT": woutT,
            "bout": bo_sb,
        })
    return in_maps


def kernel_run(inputs, trace=False, tmpdir=None):
    nc = _get_nc()
    in_maps = _host_prep(**inputs)
    res = run_bass_kernel_spmd(
        nc, in_maps, list(range(NC)), trace=trace, tmpdir=tmpdir)
    outT = np.concatenate(
        [np.asarray(res.results[c]["out"], dtype=np.float32) for c in range(NC)],
        axis=1)                                             # [1024, 4096]
    out = np.ascontiguousarray(outT.T).reshape(B, L, D)
    return out, res


def kernel(**inputs):
    out, _ = kernel_run(inputs, trace=False)
    return out
